# revision 2
# baseline (speedup 1.0000x reference)
"""2x2/stride-2 max-pool (NCHW, padding=0) on Trainium2, data-parallel over 8 cores.

Problem: x (32, 96, 224, 224) fp32 -> out (32, 96, 112, 112) fp32.

Sharding: pure data parallel on the batch dim — core i handles x[4i:4i+4].
Per core the (4, 96, 224, 224) shard is viewed as 43008 row-pairs of 448
contiguous floats ((n,c,h-pair) x (2 rows * 224 cols)).  43008 = 21 * 128 * 16,
so each of 21 iterations loads a fully contiguous [128 partitions x 16
row-pairs] block (3.5 MiB), reduces it in two elementwise-max stages
(vertical rows, then horizontal column pairs), and stores a fully contiguous
[128 x 16*112] block (0.875 MiB).
"""

import numpy as np

N_CORES = 8
M = 16          # row-pairs per partition per iteration
T = 21          # iterations per core: 43008 pairs / (128 * M)
IN_SHAPE = (32, 96, 224, 224)
H_OUT = 112

_cache = {}


def _build():
    import concourse.bass as bass  # noqa: F401
    import concourse.tile as tile
    from concourse import bacc, mybir

    nc = bacc.Bacc("TRN2", target_bir_lowering=False, debug=False)
    x = nc.dram_tensor("x", [T, 128, M * 448], mybir.dt.float32, kind="ExternalInput")
    o = nc.dram_tensor("o", [T, 128, M * 112], mybir.dt.float32, kind="ExternalOutput")
    xap, oap = x.ap(), o.ap()

    with tile.TileContext(nc) as tc:
        with (
            tc.tile_pool(name="inp", bufs=3) as pin,
            tc.tile_pool(name="v", bufs=2) as pv,
            tc.tile_pool(name="outp", bufs=2) as po,
        ):
            for t in range(T):
                tin = pin.tile([128, M, 2, 224], mybir.dt.float32)
                nc.sync.dma_start(out=tin[:], in_=xap[t])
                v = pv.tile([128, M, 112, 2], mybir.dt.float32)
                nc.any.tensor_max(
                    v.rearrange("p m a b -> p m (a b)"), tin[:, :, 0], tin[:, :, 1]
                )
                to = po.tile([128, M, 112], mybir.dt.float32)
                nc.any.tensor_max(to[:], v[:, :, :, 0], v[:, :, :, 1])
                nc.sync.dma_start(out=oap[t], in_=to[:])
    nc.compile()
    return nc


def get_nc():
    if "nc" not in _cache:
        _cache["nc"] = _build()
    return _cache["nc"]


def shard(x: np.ndarray, c: int) -> dict:
    per = IN_SHAPE[0] // N_CORES
    return {
        "x": np.ascontiguousarray(x[c * per : (c + 1) * per]).reshape(T, 128, M * 448)
    }


def unshard(outs: list) -> np.ndarray:
    per = IN_SHAPE[0] // N_CORES
    return np.concatenate(
        [o.reshape(per, IN_SHAPE[1], H_OUT, H_OUT) for o in outs], axis=0
    )


def kernel(x: np.ndarray) -> np.ndarray:
    from concourse.bass_utils import run_bass_kernel_spmd

    assert x.shape == IN_SHAPE and x.dtype == np.float32, (x.shape, x.dtype)
    nc = get_nc()
    in_maps = [shard(x, c) for c in range(N_CORES)]
    res = run_bass_kernel_spmd(nc, in_maps, list(range(N_CORES)))
    return unshard([res.results[c]["o"] for c in range(N_CORES)])
